# revision 25
# baseline (speedup 1.0000x reference)
"""Additive-attention kernel for Trainium2 (8 NeuronCores, data-parallel over batch).

Reference (B=32, S=4096, D=512, H=512):
    inp  = input @ W_in.T + b_in                       # [B, H]
    ctx  = einsum('bsd,hd->bhs', context, W_ctx) + b_ctx
    att  = einsum('h,bhs->bs', V, tanh(inp + ctx))
    att  = where(mask, -inf, att); alpha = softmax(att, axis=1)
    hidden = einsum('bhs,bs->bh', ctx, alpha)
    return hidden, alpha + 1e-6

Device strategy (per core, 4 batches):
  - DMA-cast context fp32->fp16 into SBUF natural tiles [128s, 512d] (SWDGE).
  - PE-transpose natural tiles -> ctxT [128d, s] chunks; main matmul
    ctx[H, S-chunk] = W_ctxT.T @ ctxT in fp16 (fp32 PSUM accumulate).
  - tanh from PSUM on ScalarE with per-partition bias (inp + b_in + b_ctx).
  - att row via M=1 matmul with V as stationary; softmax computed in a
    partition-spread [128, 32] layout; the whole softmax chain of batch b
    is emitted in segments interleaved with batch b+1's chunks so the PE
    never waits on the Vector/Scalar softmax latency chain.
  - hidden via the algebraic identity
        hidden = W_ctx @ (context.T @ alpha) + b_ctx
    so the big [B,H,S] ctx tensor is never re-read: weighted context is a
    matmul of unnormalized exp against the natural fp16 context tiles,
    normalized by 1/sum(exp) afterwards.
All matmuls run in fp16 (fp32 accumulate): measured end-to-end absmax
relative error ~1.3e-3 for both outputs.
"""

import sys

if "/opt/trn_rl_repo" not in sys.path:
    sys.path.insert(0, "/opt/trn_rl_repo")

import numpy as np

B, S, D, H = 32, 4096, 512, 512
NCORES = 8
BL = B // NCORES          # batches per core = 4
ST = S // 128             # s-tiles per batch = 32
NQ = 4                    # context DMA split (quarters)
QT = ST // NQ             # s-tiles per quarter = 8
DT = D // 128             # d-tiles = 4
HT = H // 128             # h-tiles = 4
SC = S // 512             # 512-wide s-chunks = 8

_NC_CACHE = []


def _build_nc():
    import concourse.bacc as bacc
    import concourse.mybir as mybir
    import concourse.tile as tile
    from concourse.masks import make_identity

    dt = mybir.dt
    AF = mybir.ActivationFunctionType

    nc = bacc.Bacc()
    nc._cats = {}

    def reg(inst, cat):
        nc._cats[inst.ins.name] = cat
        return inst

    ctx_d = nc.declare_dram_parameter("ctx", [BL, S, D], dt.float32, isOutput=False)
    maskT_d = nc.declare_dram_parameter("maskT", [BL, 128, ST], dt.float32, isOutput=False)
    wct_d = nc.declare_dram_parameter("wct", [D, H], dt.float16, isOutput=False)
    wit_d = nc.declare_dram_parameter("wit", [D, H], dt.float16, isOutput=False)
    inpT_d = nc.declare_dram_parameter("inpT", [D, BL], dt.float16, isOutput=False)
    v_d = nc.declare_dram_parameter("v", [128, HT], dt.float16, isOutput=False)
    bc2_d = nc.declare_dram_parameter("bc2", [128, HT], dt.float32, isOutput=False)
    bcr_d = nc.declare_dram_parameter("bcr", [1, H], dt.float16, isOutput=False)
    hid_d = nc.declare_dram_parameter("hidden", [BL, H], dt.float32, isOutput=True)
    alp_d = nc.declare_dram_parameter("alpha", [BL, S], dt.float32, isOutput=True)

    with tile.TileContext(nc) as tc:
        with (
            tc.tile_pool(name="const", bufs=1) as cpool,
            tc.tile_pool(name="nat", bufs=3) as natpool,
            tc.tile_pool(name="work", bufs=2) as wpool,
            tc.tile_pool(name="ctxT", bufs=4) as ctxTpool,
            tc.tile_pool(name="th", bufs=3) as thpool,
            tc.tile_pool(name="psT", bufs=2, space="PSUM") as psTpool,
            tc.tile_pool(name="mm", bufs=3, space="PSUM") as mmpool,
            tc.tile_pool(name="attc", bufs=1, space="PSUM") as attcpool,
            tc.tile_pool(name="tail", bufs=2, space="PSUM") as tailpool,
        ):
            def emit_nat_dmas(b):
                nats = []
                for q in range(NQ):
                    natq = natpool.tile([128, QT, D], dt.float16, tag=f"nat{q}")
                    nc.gpsimd.dma_start(
                        natq[:],
                        ctx_d.ap()[b, 128 * QT * q:128 * QT * (q + 1), :]
                        .rearrange("(t p) d -> p t d", p=128),
                    )
                    nats.append(natq)
                return nats

            # ---- constants ----
            id16 = cpool.tile([128, 128], dt.float16, tag="id16")
            make_identity(nc, id16[:])

            def emit_nat_dmas_first():
                # batch 0: only s-tiles 0-7 load up front; the rest trickle in
                # from the chunk loop so the first MB isn't starved by SDMA
                # round-robin across all of batch 0's queues
                nats = []
                deferred = []
                for q in range(NQ):
                    natq = natpool.tile([128, QT, D], dt.float16, tag=f"nat{q}",
                                        name=f"nat{q}")
                    if q == 0:
                        half = QT // 2
                        for hh in range(2):
                            nc.gpsimd.dma_start(
                                natq[:, half * hh:half * (hh + 1), :],
                                ctx_d.ap()[0, 128 * half * hh:128 * half * (hh + 1), :]
                                .rearrange("(t p) d -> p t d", p=128),
                            )
                    else:
                        def mk(natq=natq, q=q):
                            def go():
                                nc.gpsimd.dma_start(
                                    natq[:],
                                    ctx_d.ap()[0, 128 * QT * q:128 * QT * (q + 1), :]
                                    .rearrange("(t p) d -> p t d", p=128),
                                )
                            return go
                        deferred.append(mk())
                    nats.append(natq)
                return nats, deferred

            nats_next, b0_deferred = emit_nat_dmas_first()
            id32f = cpool.tile([32, 32], dt.float32, tag="id32f")
            make_identity(nc, id32f[:])
            id4 = cpool.tile([4, 4], dt.float16, tag="id4")
            make_identity(nc, id4[:])
            id128f = cpool.tile([128, 128], dt.float32, tag="id128f")
            make_identity(nc, id128f[:])
            ones_row = cpool.tile([1, 128], dt.float16, tag="ones_row")
            nc.gpsimd.memset(ones_row[:], 1.0)
            ones_col = cpool.tile([128, 1], dt.float16, tag="ones_col")
            nc.gpsimd.memset(ones_col[:], 1.0)

            wct_sb = cpool.tile([128, DT, H], dt.float16, tag="wct")
            nc.sync.dma_start(wct_sb[:], wct_d.ap().rearrange("(t p) h -> p t h", p=128))
            wit_sb = cpool.tile([128, DT, H], dt.float16, tag="wit")
            nc.sync.dma_start(wit_sb[:], wit_d.ap().rearrange("(t p) h -> p t h", p=128))
            inpT_sb = cpool.tile([128, DT, BL], dt.float16, tag="inpT")
            nc.sync.dma_start(inpT_sb[:], inpT_d.ap().rearrange("(t p) b -> p t b", p=128))
            v_sb = cpool.tile([128, HT], dt.float16, tag="v")
            nc.sync.dma_start(v_sb[:], v_d.ap())
            bc2_sb = cpool.tile([128, HT], dt.float32, tag="bc2")
            nc.sync.dma_start(bc2_sb[:], bc2_d.ap())
            bcr_sb = cpool.tile([1, H], dt.float16, tag="bcr")
            nc.sync.dma_start(bcr_sb[:], bcr_d.ap())
            maskT_sb = cpool.tile([128, BL, ST], dt.float32, tag="maskT")
            nc.sync.dma_start(maskT_sb[:], maskT_d.ap().rearrange("b p k -> p b k"))

            # ---- bias_sb[p, ht, b] = inp[b, 128*ht+p] + b_in[.] + b_ctx[.]
            # (emitted after the first transposes so PE can start immediately)
            bias_sb = cpool.tile([128, HT, BL], dt.float32, tag="bias")

            def emit_bias():
                bias_ps = tailpool.tile([128, HT, BL], dt.float32, tag="tail", name="bias_ps")
                for mt in range(HT):
                    for kt in range(DT):
                        nc.tensor.matmul(
                            bias_ps[:, mt, :],
                            wit_sb[:, kt, 128 * mt:128 * (mt + 1)],
                            inpT_sb[:, kt, :],
                            start=(kt == 0), stop=(kt == DT - 1),
                        )
                for mt in range(HT):
                    nc.scalar.activation(
                        bias_sb[:, mt, :], bias_ps[:, mt, :],
                        AF.Identity, bias=bc2_sb[:, mt:mt + 1],
                    )

            wcnT_all = cpool.tile([128, DT, BL], dt.float16, tag="wcnT")

            # ---- deferred softmax/weighted-sum tail, emitted in segments ----
            def tail_seg0(st):
                b = st["b"]
                st["attn32"] = wpool.tile([32, 128], dt.float32, tag="attn32", name="attn32")
                nc.sync.dma_start(st["attn32"][:], st["att_row"][0:1, :])
                st["attT_ps"] = tailpool.tile([128, 32], dt.float32, tag="tail", name="attT_ps")
                nc.tensor.transpose(st["attT_ps"][:], st["attn32"][:], id32f[:])
                st["attm"] = wpool.tile([128, ST], dt.float32, tag="attm", name="attm")
                colmax = wpool.tile([128, 1], dt.float32, tag="colmax")
                nc.vector.tensor_tensor(
                    st["attm"][:], st["attT_ps"][:], maskT_sb[:, b, :],
                    mybir.AluOpType.add,
                )
                nc.vector.tensor_reduce(
                    colmax[:], st["attm"][:], axis=mybir.AxisListType.X,
                    op=mybir.AluOpType.max,
                )
                st["colmax16"] = wpool.tile([128, 1], dt.float16, tag="colmax16", name="colmax16")
                nc.vector.tensor_copy(st["colmax16"][:], colmax[:])

            def tail_seg1(st):
                cm_ps = tailpool.tile([1, 128], dt.float16, tag="tail")
                nc.tensor.transpose(cm_ps[:], st["colmax16"][:], id16[:])
                negmax16 = wpool.tile([1, 1], dt.float16, tag="negmax16")
                nc.vector.tensor_reduce(
                    negmax16[:], cm_ps[:], axis=mybir.AxisListType.X,
                    op=mybir.AluOpType.max, negate=True,
                )
                nm_ps = tailpool.tile([128, 1], dt.float32, tag="tail")
                nc.tensor.matmul(nm_ps[:], ones_row[:], negmax16[:], start=True, stop=True)
                st["nm_sb"] = wpool.tile([128, 1], dt.float32, tag="nm_sb", name="nm_sb")
                nc.vector.tensor_copy(st["nm_sb"][:], nm_ps[:])
                st["exp_t"] = wpool.tile([128, ST], dt.float16, tag="exp_t", name="exp_t")
                nc.scalar.activation(st["exp_t"][:], st["attm"][:], AF.Exp, bias=st["nm_sb"][:])

            def tail_seg2(st):
                cs_ps = tailpool.tile([1, ST], dt.float32, tag="tail")
                nc.tensor.matmul(cs_ps[:], ones_col[:], st["exp_t"][:], start=True, stop=True)
                sumexp = wpool.tile([1, 1], dt.float32, tag="sumexp")
                nc.vector.tensor_reduce(
                    sumexp[:], cs_ps[:], axis=mybir.AxisListType.X,
                    op=mybir.AluOpType.add,
                )
                st["recip"] = wpool.tile([1, 1], dt.float32, tag="recip", name="recip")
                nc.vector.reciprocal(st["recip"][:], sumexp[:])
                st["recip16"] = wpool.tile([1, 1], dt.float16, tag="recip16", name="recip16")
                nc.vector.tensor_copy(st["recip16"][:], st["recip"][:])

            def tail_seg3(st):
                b, nats, exp_t = st["b"], st["nats"], st["exp_t"]
                et_ps = tailpool.tile([32, 128], dt.float16, tag="tail")
                nc.tensor.transpose(et_ps[:], exp_t[:], id16[:])
                rb_ps = tailpool.tile([128, 1], dt.float32, tag="tail")
                nc.tensor.matmul(rb_ps[:], ones_row[:], st["recip16"][:], start=True, stop=True)
                alpha32 = wpool.tile([32, 128], dt.float32, tag="alpha32")
                nc.vector.tensor_scalar(
                    out=alpha32[:], in0=et_ps[:],
                    scalar1=rb_ps[0:32, :], scalar2=1e-6,
                    op0=mybir.AluOpType.mult, op1=mybir.AluOpType.add,
                )
                nc.sync.dma_start(
                    alp_d.ap()[b].rearrange("(k p) -> k p", k=32), alpha32[:]
                )
                # weighted context: wc = sum_s exp[s] * context[b, s, :]
                wc_ps = tailpool.tile([1, D], dt.float32, tag="tail")
                for sti in range(ST):
                    reg(nc.tensor.matmul(
                        wc_ps[:], exp_t[:, sti:sti + 1], nats[sti // QT][:, sti % QT, :],
                        start=(sti == 0), stop=(sti == ST - 1),
                    ), "wsum")
                wcn16 = wpool.tile([1, D], dt.float16, tag="wcn16")
                nc.vector.tensor_scalar(
                    out=wcn16[:], in0=wc_ps[:], scalar1=st["recip"][:], scalar2=None,
                    op0=mybir.AluOpType.mult,
                )
                wcn4 = wpool.tile([4, 128], dt.float16, tag="wcn4")
                nc.sync.dma_start(wcn4[:], wcn16[:])
                wcnT_ps = tailpool.tile([128, 4], dt.float16, tag="tail")
                nc.tensor.transpose(wcnT_ps[:], wcn4[:], id4[:])
                nc.vector.tensor_copy(wcnT_all[:, :, b], wcnT_ps[:])

            tail_segs = [tail_seg0, tail_seg1, tail_seg2, tail_seg3]
            pending = None

            for b in range(BL):
                nats = nats_next
                att_row = wpool.tile([1, S], dt.float32, tag="att_row")

                def emit_att_group(sc, th):
                    attc = attcpool.tile([1, 512], dt.float32, tag="attc", name="attc")
                    for ht in range(HT):
                        reg(nc.tensor.matmul(
                            attc[:], v_sb[:, ht:ht + 1], th[:, ht, :],
                            start=(ht == 0), stop=(ht == HT - 1),
                        ), "att")
                    nc.scalar.copy(att_row[0:1, 512 * sc:512 * (sc + 1)], attc[:])

                pending_att = None
                for sc in range(SC):
                    # transpose 4 s-tiles x 4 d-blocks -> ctxT [128d, DT, 512s]
                    ctxT = ctxTpool.tile([128, DT, 512], dt.float16, tag="ctxT")
                    for dtb in range(DT):
                        psT = psTpool.tile([128, 512], dt.float16, tag="psT")
                        for j in range(4):
                            sti = 4 * sc + j
                            reg(nc.tensor.transpose(
                                psT[:, 128 * j:128 * (j + 1)],
                                nats[sti // QT][:, sti % QT, 128 * dtb:128 * (dtb + 1)],
                                id16[:],
                            ), "T")
                        nc.vector.tensor_copy(ctxT[:, dtb, :], psT[:])

                    if b == 0 and sc == 0:
                        emit_bias()

                    th = thpool.tile([128, HT, 512], dt.float16, tag="th")
                    for ht in range(HT):
                        mmps = mmpool.tile([128, 512], dt.float32, tag="mm")
                        for dtb in range(DT):
                            reg(nc.tensor.matmul(
                                mmps[:],
                                wct_sb[:, dtb, 128 * ht:128 * (ht + 1)],
                                ctxT[:, dtb, :],
                                start=(dtb == 0), stop=(dtb == DT - 1),
                            ), "main")
                        nc.scalar.activation(
                            th[:, ht, :], mmps[:], AF.Tanh,
                            bias=bias_sb[:, ht, b:b + 1],
                        )
                    if pending_att is not None:
                        emit_att_group(*pending_att)
                    pending_att = (sc, th)

                    # interleave the previous batch's softmax/wsum tail
                    if pending is not None and 1 <= sc <= len(tail_segs):
                        tail_segs[sc - 1](pending)
                        if sc == len(tail_segs):
                            pending = None
                    if b == 0 and sc < len(b0_deferred):
                        b0_deferred[sc]()
                    if sc == 1 and b + 1 < BL:
                        nats_next = emit_nat_dmas(b + 1)

                emit_att_group(*pending_att)
                pending = {"b": b, "nats": nats, "att_row": att_row}

            for seg in tail_segs:
                seg(pending)

            # ---- hidden = W_ctx @ wcn + b_ctx, all 4 batches at once ----
            hid_ps = tailpool.tile([128, HT, BL], dt.float32, tag="tail")
            for mt in range(HT):
                for kt in range(DT):
                    nc.tensor.matmul(
                        hid_ps[:, mt, :],
                        wct_sb[:, kt, 128 * mt:128 * (mt + 1)],
                        wcnT_all[:, kt, :],
                        start=(kt == 0), stop=False,
                    )
                nc.tensor.matmul(
                    hid_ps[:, mt, :], bcr_sb[0:1, 128 * mt:128 * (mt + 1)],
                    ones_row[0:1, 0:BL],
                    start=False, stop=True,
                )
            hid_sb = wpool.tile([128, HT, BL], dt.float32, tag="hid_sb")
            nc.vector.tensor_copy(hid_sb[:], hid_ps[:])
            hidT_ps = tailpool.tile([HT * BL, 128], dt.float32, tag="tail")
            nc.tensor.transpose(
                hidT_ps[:], hid_sb[:].rearrange("p t b -> p (t b)"), id128f[:]
            )
            hidT_sb = wpool.tile([HT * BL, 128], dt.float32, tag="hidT_sb")
            nc.vector.tensor_copy(hidT_sb[:], hidT_ps[:])
            nc.sync.dma_start(
                hid_d.ap().rearrange("b (t p) -> t b p", p=128), hidT_sb[:]
            )

    nc.finalize()
    return nc


def _get_nc():
    if not _NC_CACHE:
        _NC_CACHE.append(_build_nc())
    return _NC_CACHE[0]


def _prepare_in_maps(input, context, mask, W_in, b_in, W_ctx, b_ctx, V):
    input = np.asarray(input, dtype=np.float32)
    context = np.asarray(context, dtype=np.float32)
    mask = np.asarray(mask)
    W_in = np.asarray(W_in, dtype=np.float32)
    b_in = np.asarray(b_in, dtype=np.float32)
    W_ctx = np.asarray(W_ctx, dtype=np.float32)
    b_ctx = np.asarray(b_ctx, dtype=np.float32)
    V = np.asarray(V, dtype=np.float32)

    # host-side prep (small tensors only)
    maskadd = np.where(mask, np.float32(-1e30), np.float32(0.0))      # [B, S]
    maskT = np.ascontiguousarray(
        maskadd.reshape(B, ST, 128).transpose(0, 2, 1))               # [B, 128, ST]
    wct16 = np.ascontiguousarray(W_ctx.T).astype(np.float16)          # [D, H]
    wit16 = np.ascontiguousarray(W_in.T).astype(np.float16)           # [D, H]
    v16 = np.ascontiguousarray(V.reshape(HT, 128).T).astype(np.float16)   # [128, HT]
    bc2 = np.ascontiguousarray((b_in + b_ctx).reshape(HT, 128).T)     # [128, HT] f32
    bcr16 = b_ctx.reshape(1, H).astype(np.float16)                    # [1, H]

    in_maps = []
    for c in range(NCORES):
        bs = slice(BL * c, BL * (c + 1))
        in_maps.append({
            "ctx": context[bs],
            "maskT": maskT[bs],
            "wct": wct16,
            "wit": wit16,
            "inpT": np.ascontiguousarray(input[bs].T).astype(np.float16),
            "v": v16,
            "bc2": bc2,
            "bcr": bcr16,
        })
    return in_maps


def kernel(input, context, mask, W_in, b_in, W_ctx, b_ctx, V, **run_kwargs):
    from concourse.bass_utils import run_bass_kernel_spmd

    nc = _get_nc()
    in_maps = _prepare_in_maps(input, context, mask, W_in, b_in, W_ctx, b_ctx, V)
    res = run_bass_kernel_spmd(nc, in_maps, list(range(NCORES)), **run_kwargs)
    hidden = np.concatenate([res.results[c]["hidden"] for c in range(NCORES)], axis=0)
    alpha = np.concatenate([res.results[c]["alpha"] for c in range(NCORES)], axis=0)
    if run_kwargs:
        kernel.last_result = res
    return hidden, alpha


# revision 26
# speedup vs baseline: 1.0262x; 1.0262x over previous
"""Additive-attention kernel for Trainium2 (8 NeuronCores, data-parallel over batch).

Reference (B=32, S=4096, D=512, H=512):
    inp  = input @ W_in.T + b_in                       # [B, H]
    ctx  = einsum('bsd,hd->bhs', context, W_ctx) + b_ctx
    att  = einsum('h,bhs->bs', V, tanh(inp + ctx))
    att  = where(mask, -inf, att); alpha = softmax(att, axis=1)
    hidden = einsum('bhs,bs->bh', ctx, alpha)
    return hidden, alpha + 1e-6

Device strategy (per core, 4 batches):
  - DMA-cast context fp32->fp16 into SBUF natural tiles [128s, 512d] (SWDGE).
  - PE-transpose natural tiles -> ctxT [128d, s] chunks; main matmul
    ctx[H, S-chunk] = W_ctxT.T @ ctxT in fp16 (fp32 PSUM accumulate).
  - tanh from PSUM on ScalarE with per-partition bias (inp + b_in + b_ctx).
  - att row via M=1 matmul with V as stationary; softmax computed in a
    partition-spread [128, 32] layout; the whole softmax chain of batch b
    is emitted in segments interleaved with batch b+1's chunks so the PE
    never waits on the Vector/Scalar softmax latency chain.
  - hidden via the algebraic identity
        hidden = W_ctx @ (context.T @ alpha) + b_ctx
    so the big [B,H,S] ctx tensor is never re-read: weighted context is a
    matmul of unnormalized exp against the natural fp16 context tiles,
    normalized by 1/sum(exp) afterwards.
All matmuls run in fp16 (fp32 accumulate): measured end-to-end absmax
relative error ~1.3e-3 for both outputs.
"""

import sys

if "/opt/trn_rl_repo" not in sys.path:
    sys.path.insert(0, "/opt/trn_rl_repo")

import numpy as np

B, S, D, H = 32, 4096, 512, 512
NCORES = 8
BL = B // NCORES          # batches per core = 4
ST = S // 128             # s-tiles per batch = 32
NQ = 4                    # context DMA split (quarters)
QT = ST // NQ             # s-tiles per quarter = 8
DT = D // 128             # d-tiles = 4
HT = H // 128             # h-tiles = 4
SC = S // 512             # 512-wide s-chunks = 8

_NC_CACHE = []


def _build_nc():
    import concourse.bacc as bacc
    import concourse.mybir as mybir
    import concourse.tile as tile
    from concourse.masks import make_identity

    dt = mybir.dt
    AF = mybir.ActivationFunctionType

    nc = bacc.Bacc()
    nc._cats = {}

    def reg(inst, cat):
        nc._cats[inst.ins.name] = cat
        return inst

    ctx_d = nc.declare_dram_parameter("ctx", [BL, S, D], dt.float32, isOutput=False)
    maskT_d = nc.declare_dram_parameter("maskT", [BL, 128, ST], dt.float32, isOutput=False)
    wct_d = nc.declare_dram_parameter("wct", [D, H], dt.float16, isOutput=False)
    wit_d = nc.declare_dram_parameter("wit", [D, H], dt.float16, isOutput=False)
    inpT_d = nc.declare_dram_parameter("inpT", [D, BL], dt.float16, isOutput=False)
    v_d = nc.declare_dram_parameter("v", [128, HT], dt.float16, isOutput=False)
    bc2_d = nc.declare_dram_parameter("bc2", [128, HT], dt.float32, isOutput=False)
    bcr_d = nc.declare_dram_parameter("bcr", [1, H], dt.float16, isOutput=False)
    hid_d = nc.declare_dram_parameter("hidden", [BL, H], dt.float32, isOutput=True)
    alp_d = nc.declare_dram_parameter("alpha", [BL, S], dt.float32, isOutput=True)

    with tile.TileContext(nc) as tc:
        with (
            tc.tile_pool(name="const", bufs=1) as cpool,
            tc.tile_pool(name="nat", bufs=3) as natpool,
            tc.tile_pool(name="work", bufs=2) as wpool,
            tc.tile_pool(name="ctxT", bufs=4) as ctxTpool,
            tc.tile_pool(name="th", bufs=3) as thpool,
            tc.tile_pool(name="psT", bufs=2, space="PSUM") as psTpool,
            tc.tile_pool(name="mm", bufs=3, space="PSUM") as mmpool,
            tc.tile_pool(name="attc", bufs=1, space="PSUM") as attcpool,
            tc.tile_pool(name="tail", bufs=2, space="PSUM") as tailpool,
        ):
            def emit_nat_dmas(b):
                nats = []
                for q in range(NQ):
                    natq = natpool.tile([128, QT, D], dt.float16, tag=f"nat{q}")
                    nc.gpsimd.dma_start(
                        natq[:],
                        ctx_d.ap()[b, 128 * QT * q:128 * QT * (q + 1), :]
                        .rearrange("(t p) d -> p t d", p=128),
                    )
                    nats.append(natq)
                return nats

            # ---- constants ----
            id16 = cpool.tile([128, 128], dt.float16, tag="id16")
            make_identity(nc, id16[:])

            # get the first context load moving before the other constants;
            # batch 0's first quarter is split so transposes can start sooner
            def emit_nat_dmas_first():
                nats = []
                for q in range(NQ):
                    natq = natpool.tile([128, QT, D], dt.float16, tag=f"nat{q}",
                                        name=f"nat{q}")
                    if q == 0:
                        half = QT // 2
                        for hh in range(2):
                            nc.gpsimd.dma_start(
                                natq[:, half * hh:half * (hh + 1), :],
                                ctx_d.ap()[0, 128 * half * hh:128 * half * (hh + 1), :]
                                .rearrange("(t p) d -> p t d", p=128),
                            )
                    else:
                        nc.gpsimd.dma_start(
                            natq[:],
                            ctx_d.ap()[0, 128 * QT * q:128 * QT * (q + 1), :]
                            .rearrange("(t p) d -> p t d", p=128),
                        )
                    nats.append(natq)
                return nats

            nats_next = emit_nat_dmas_first()
            id32f = cpool.tile([32, 32], dt.float32, tag="id32f")
            make_identity(nc, id32f[:])
            id4 = cpool.tile([4, 4], dt.float16, tag="id4")
            make_identity(nc, id4[:])
            id128f = cpool.tile([128, 128], dt.float32, tag="id128f")
            make_identity(nc, id128f[:])
            ones_row = cpool.tile([1, 128], dt.float16, tag="ones_row")
            nc.gpsimd.memset(ones_row[:], 1.0)
            ones_col = cpool.tile([128, 1], dt.float16, tag="ones_col")
            nc.gpsimd.memset(ones_col[:], 1.0)

            wct_sb = cpool.tile([128, DT, H], dt.float16, tag="wct")
            nc.sync.dma_start(wct_sb[:], wct_d.ap().rearrange("(t p) h -> p t h", p=128))
            wit_sb = cpool.tile([128, DT, H], dt.float16, tag="wit")
            nc.sync.dma_start(wit_sb[:], wit_d.ap().rearrange("(t p) h -> p t h", p=128))
            inpT_sb = cpool.tile([128, DT, BL], dt.float16, tag="inpT")
            nc.sync.dma_start(inpT_sb[:], inpT_d.ap().rearrange("(t p) b -> p t b", p=128))
            v_sb = cpool.tile([128, HT], dt.float16, tag="v")
            nc.sync.dma_start(v_sb[:], v_d.ap())
            bc2_sb = cpool.tile([128, HT], dt.float32, tag="bc2")
            nc.sync.dma_start(bc2_sb[:], bc2_d.ap())
            bcr_sb = cpool.tile([1, H], dt.float16, tag="bcr")
            nc.sync.dma_start(bcr_sb[:], bcr_d.ap())
            maskT_sb = cpool.tile([128, BL, ST], dt.float32, tag="maskT")
            nc.sync.dma_start(maskT_sb[:], maskT_d.ap().rearrange("b p k -> p b k"))

            # ---- bias_sb[p, ht, b] = inp[b, 128*ht+p] + b_in[.] + b_ctx[.]
            # (emitted after the first transposes so PE can start immediately)
            bias_sb = cpool.tile([128, HT, BL], dt.float32, tag="bias")

            def emit_bias():
                bias_ps = tailpool.tile([128, HT, BL], dt.float32, tag="tail", name="bias_ps")
                for mt in range(HT):
                    for kt in range(DT):
                        nc.tensor.matmul(
                            bias_ps[:, mt, :],
                            wit_sb[:, kt, 128 * mt:128 * (mt + 1)],
                            inpT_sb[:, kt, :],
                            start=(kt == 0), stop=(kt == DT - 1),
                        )
                for mt in range(HT):
                    nc.scalar.activation(
                        bias_sb[:, mt, :], bias_ps[:, mt, :],
                        AF.Identity, bias=bc2_sb[:, mt:mt + 1],
                    )

            wcnT_all = cpool.tile([128, DT, BL], dt.float16, tag="wcnT")

            # ---- deferred softmax/weighted-sum tail, emitted in segments ----
            def tail_seg0(st):
                b = st["b"]
                st["attn32"] = wpool.tile([32, 128], dt.float32, tag="attn32", name="attn32")
                nc.sync.dma_start(st["attn32"][:], st["att_row"][0:1, :])
                st["attT_ps"] = tailpool.tile([128, 32], dt.float32, tag="tail", name="attT_ps")
                nc.tensor.transpose(st["attT_ps"][:], st["attn32"][:], id32f[:])
                st["attm"] = wpool.tile([128, ST], dt.float32, tag="attm", name="attm")
                colmax = wpool.tile([128, 1], dt.float32, tag="colmax")
                nc.vector.tensor_tensor(
                    st["attm"][:], st["attT_ps"][:], maskT_sb[:, b, :],
                    mybir.AluOpType.add,
                )
                nc.vector.tensor_reduce(
                    colmax[:], st["attm"][:], axis=mybir.AxisListType.X,
                    op=mybir.AluOpType.max,
                )
                st["colmax16"] = wpool.tile([128, 1], dt.float16, tag="colmax16", name="colmax16")
                nc.vector.tensor_copy(st["colmax16"][:], colmax[:])

            def tail_seg1(st):
                cm_ps = tailpool.tile([1, 128], dt.float16, tag="tail")
                nc.tensor.transpose(cm_ps[:], st["colmax16"][:], id16[:])
                negmax16 = wpool.tile([1, 1], dt.float16, tag="negmax16")
                nc.vector.tensor_reduce(
                    negmax16[:], cm_ps[:], axis=mybir.AxisListType.X,
                    op=mybir.AluOpType.max, negate=True,
                )
                nm_ps = tailpool.tile([128, 1], dt.float32, tag="tail")
                nc.tensor.matmul(nm_ps[:], ones_row[:], negmax16[:], start=True, stop=True)
                st["nm_sb"] = wpool.tile([128, 1], dt.float32, tag="nm_sb", name="nm_sb")
                nc.vector.tensor_copy(st["nm_sb"][:], nm_ps[:])
                st["exp_t"] = wpool.tile([128, ST], dt.float16, tag="exp_t", name="exp_t")
                nc.scalar.activation(st["exp_t"][:], st["attm"][:], AF.Exp, bias=st["nm_sb"][:])

            def tail_seg2(st):
                cs_ps = tailpool.tile([1, ST], dt.float32, tag="tail")
                nc.tensor.matmul(cs_ps[:], ones_col[:], st["exp_t"][:], start=True, stop=True)
                sumexp = wpool.tile([1, 1], dt.float32, tag="sumexp")
                nc.vector.tensor_reduce(
                    sumexp[:], cs_ps[:], axis=mybir.AxisListType.X,
                    op=mybir.AluOpType.add,
                )
                st["recip"] = wpool.tile([1, 1], dt.float32, tag="recip", name="recip")
                nc.vector.reciprocal(st["recip"][:], sumexp[:])
                st["recip16"] = wpool.tile([1, 1], dt.float16, tag="recip16", name="recip16")
                nc.vector.tensor_copy(st["recip16"][:], st["recip"][:])

            def tail_seg3(st):
                b, nats, exp_t = st["b"], st["nats"], st["exp_t"]
                et_ps = tailpool.tile([32, 128], dt.float16, tag="tail")
                nc.tensor.transpose(et_ps[:], exp_t[:], id16[:])
                rb_ps = tailpool.tile([128, 1], dt.float32, tag="tail")
                nc.tensor.matmul(rb_ps[:], ones_row[:], st["recip16"][:], start=True, stop=True)
                alpha32 = wpool.tile([32, 128], dt.float32, tag="alpha32")
                nc.vector.tensor_scalar(
                    out=alpha32[:], in0=et_ps[:],
                    scalar1=rb_ps[0:32, :], scalar2=1e-6,
                    op0=mybir.AluOpType.mult, op1=mybir.AluOpType.add,
                )
                nc.sync.dma_start(
                    alp_d.ap()[b].rearrange("(k p) -> k p", k=32), alpha32[:]
                )
                # weighted context: wc = sum_s exp[s] * context[b, s, :]
                wc_ps = tailpool.tile([1, D], dt.float32, tag="tail")
                for sti in range(ST):
                    reg(nc.tensor.matmul(
                        wc_ps[:], exp_t[:, sti:sti + 1], nats[sti // QT][:, sti % QT, :],
                        start=(sti == 0), stop=(sti == ST - 1),
                    ), "wsum")
                wcn16 = wpool.tile([1, D], dt.float16, tag="wcn16")
                nc.vector.tensor_scalar(
                    out=wcn16[:], in0=wc_ps[:], scalar1=st["recip"][:], scalar2=None,
                    op0=mybir.AluOpType.mult,
                )
                wcn4 = wpool.tile([4, 128], dt.float16, tag="wcn4")
                nc.sync.dma_start(wcn4[:], wcn16[:])
                wcnT_ps = tailpool.tile([128, 4], dt.float16, tag="tail")
                nc.tensor.transpose(wcnT_ps[:], wcn4[:], id4[:])
                nc.vector.tensor_copy(wcnT_all[:, :, b], wcnT_ps[:])

            tail_segs = [tail_seg0, tail_seg1, tail_seg2, tail_seg3]
            pending = None

            for b in range(BL):
                nats = nats_next
                att_row = wpool.tile([1, S], dt.float32, tag="att_row")

                def emit_att_group(sc, th):
                    attc = attcpool.tile([1, 512], dt.float32, tag="attc", name="attc")
                    for ht in range(HT):
                        reg(nc.tensor.matmul(
                            attc[:], v_sb[:, ht:ht + 1], th[:, ht, :],
                            start=(ht == 0), stop=(ht == HT - 1),
                        ), "att")
                    nc.scalar.copy(att_row[0:1, 512 * sc:512 * (sc + 1)], attc[:])

                pending_att = None
                for sc in range(SC):
                    # transpose 4 s-tiles x 4 d-blocks -> ctxT [128d, DT, 512s]
                    ctxT = ctxTpool.tile([128, DT, 512], dt.float16, tag="ctxT")
                    for dtb in range(DT):
                        psT = psTpool.tile([128, 512], dt.float16, tag="psT")
                        for j in range(4):
                            sti = 4 * sc + j
                            reg(nc.tensor.transpose(
                                psT[:, 128 * j:128 * (j + 1)],
                                nats[sti // QT][:, sti % QT, 128 * dtb:128 * (dtb + 1)],
                                id16[:],
                            ), "T")
                        nc.vector.tensor_copy(ctxT[:, dtb, :], psT[:])

                    if b == 0 and sc == 0:
                        emit_bias()

                    th = thpool.tile([128, HT, 512], dt.float16, tag="th")
                    for ht in range(HT):
                        mmps = mmpool.tile([128, 512], dt.float32, tag="mm")
                        for dtb in range(DT):
                            reg(nc.tensor.matmul(
                                mmps[:],
                                wct_sb[:, dtb, 128 * ht:128 * (ht + 1)],
                                ctxT[:, dtb, :],
                                start=(dtb == 0), stop=(dtb == DT - 1),
                            ), "main")
                        nc.scalar.activation(
                            th[:, ht, :], mmps[:], AF.Tanh,
                            bias=bias_sb[:, ht, b:b + 1],
                        )
                    if pending_att is not None:
                        emit_att_group(*pending_att)
                    pending_att = (sc, th)

                    # interleave the previous batch's softmax/wsum tail
                    if pending is not None and 1 <= sc <= len(tail_segs):
                        tail_segs[sc - 1](pending)
                        if sc == len(tail_segs):
                            pending = None
                    if sc == 1 and b + 1 < BL:
                        nats_next = emit_nat_dmas(b + 1)

                emit_att_group(*pending_att)
                pending = {"b": b, "nats": nats, "att_row": att_row}

            for seg in tail_segs:
                seg(pending)

            # ---- hidden = W_ctx @ wcn + b_ctx, all 4 batches at once ----
            hid_ps = tailpool.tile([128, HT, BL], dt.float32, tag="tail")
            for mt in range(HT):
                for kt in range(DT):
                    nc.tensor.matmul(
                        hid_ps[:, mt, :],
                        wct_sb[:, kt, 128 * mt:128 * (mt + 1)],
                        wcnT_all[:, kt, :],
                        start=(kt == 0), stop=False,
                    )
                nc.tensor.matmul(
                    hid_ps[:, mt, :], bcr_sb[0:1, 128 * mt:128 * (mt + 1)],
                    ones_row[0:1, 0:BL],
                    start=False, stop=True,
                )
            hid_sb = wpool.tile([128, HT, BL], dt.float32, tag="hid_sb")
            nc.vector.tensor_copy(hid_sb[:], hid_ps[:])
            hidT_ps = tailpool.tile([HT * BL, 128], dt.float32, tag="tail")
            nc.tensor.transpose(
                hidT_ps[:], hid_sb[:].rearrange("p t b -> p (t b)"), id128f[:]
            )
            hidT_sb = wpool.tile([HT * BL, 128], dt.float32, tag="hidT_sb")
            nc.vector.tensor_copy(hidT_sb[:], hidT_ps[:])
            nc.sync.dma_start(
                hid_d.ap().rearrange("b (t p) -> t b p", p=128), hidT_sb[:]
            )

    nc.finalize()
    return nc


def _get_nc():
    if not _NC_CACHE:
        _NC_CACHE.append(_build_nc())
    return _NC_CACHE[0]


def _prepare_in_maps(input, context, mask, W_in, b_in, W_ctx, b_ctx, V):
    input = np.asarray(input, dtype=np.float32)
    context = np.asarray(context, dtype=np.float32)
    mask = np.asarray(mask)
    W_in = np.asarray(W_in, dtype=np.float32)
    b_in = np.asarray(b_in, dtype=np.float32)
    W_ctx = np.asarray(W_ctx, dtype=np.float32)
    b_ctx = np.asarray(b_ctx, dtype=np.float32)
    V = np.asarray(V, dtype=np.float32)

    # host-side prep (small tensors only)
    maskadd = np.where(mask, np.float32(-1e30), np.float32(0.0))      # [B, S]
    maskT = np.ascontiguousarray(
        maskadd.reshape(B, ST, 128).transpose(0, 2, 1))               # [B, 128, ST]
    wct16 = np.ascontiguousarray(W_ctx.T).astype(np.float16)          # [D, H]
    wit16 = np.ascontiguousarray(W_in.T).astype(np.float16)           # [D, H]
    v16 = np.ascontiguousarray(V.reshape(HT, 128).T).astype(np.float16)   # [128, HT]
    bc2 = np.ascontiguousarray((b_in + b_ctx).reshape(HT, 128).T)     # [128, HT] f32
    bcr16 = b_ctx.reshape(1, H).astype(np.float16)                    # [1, H]

    in_maps = []
    for c in range(NCORES):
        bs = slice(BL * c, BL * (c + 1))
        in_maps.append({
            "ctx": context[bs],
            "maskT": maskT[bs],
            "wct": wct16,
            "wit": wit16,
            "inpT": np.ascontiguousarray(input[bs].T).astype(np.float16),
            "v": v16,
            "bc2": bc2,
            "bcr": bcr16,
        })
    return in_maps


def kernel(input, context, mask, W_in, b_in, W_ctx, b_ctx, V, **run_kwargs):
    from concourse.bass_utils import run_bass_kernel_spmd

    nc = _get_nc()
    in_maps = _prepare_in_maps(input, context, mask, W_in, b_in, W_ctx, b_ctx, V)
    res = run_bass_kernel_spmd(nc, in_maps, list(range(NCORES)), **run_kwargs)
    hidden = np.concatenate([res.results[c]["hidden"] for c in range(NCORES)], axis=0)
    alpha = np.concatenate([res.results[c]["alpha"] for c in range(NCORES)], axis=0)
    if run_kwargs:
        kernel.last_result = res
    return hidden, alpha


# revision 27
# speedup vs baseline: 1.0538x; 1.0269x over previous
"""Additive-attention kernel for Trainium2 (8 NeuronCores, data-parallel over batch).

Reference (B=32, S=4096, D=512, H=512):
    inp  = input @ W_in.T + b_in                       # [B, H]
    ctx  = einsum('bsd,hd->bhs', context, W_ctx) + b_ctx
    att  = einsum('h,bhs->bs', V, tanh(inp + ctx))
    att  = where(mask, -inf, att); alpha = softmax(att, axis=1)
    hidden = einsum('bhs,bs->bh', ctx, alpha)
    return hidden, alpha + 1e-6

Device strategy (per core, 4 batches):
  - DMA-cast context fp32->fp16 into SBUF natural tiles [128s, 512d] (SWDGE).
  - PE-transpose natural tiles -> ctxT [128d, s] chunks; main matmul
    ctx[H, S-chunk] = W_ctxT.T @ ctxT in fp16 (fp32 PSUM accumulate).
  - tanh from PSUM on ScalarE with per-partition bias (inp + b_in + b_ctx).
  - att row via M=1 matmul with V as stationary; softmax computed in a
    partition-spread [128, 32] layout; the whole softmax chain of batch b
    is emitted in segments interleaved with batch b+1's chunks so the PE
    never waits on the Vector/Scalar softmax latency chain.
  - hidden via the algebraic identity
        hidden = W_ctx @ (context.T @ alpha) + b_ctx
    so the big [B,H,S] ctx tensor is never re-read: weighted context is a
    matmul of unnormalized exp against the natural fp16 context tiles,
    normalized by 1/sum(exp) afterwards.
All matmuls run in fp16 (fp32 accumulate): measured end-to-end absmax
relative error ~1.3e-3 for both outputs.
"""

import sys

if "/opt/trn_rl_repo" not in sys.path:
    sys.path.insert(0, "/opt/trn_rl_repo")

import numpy as np

B, S, D, H = 32, 4096, 512, 512
NCORES = 8
BL = B // NCORES          # batches per core = 4
ST = S // 128             # s-tiles per batch = 32
NQ = 4                    # context DMA split (quarters)
QT = ST // NQ             # s-tiles per quarter = 8
DT = D // 128             # d-tiles = 4
HT = H // 128             # h-tiles = 4
SC = S // 512             # 512-wide s-chunks = 8

_NC_CACHE = []


def _build_nc():
    import concourse.bacc as bacc
    import concourse.mybir as mybir
    import concourse.tile as tile
    from concourse.masks import make_identity

    dt = mybir.dt
    AF = mybir.ActivationFunctionType

    nc = bacc.Bacc()
    nc._cats = {}

    def reg(inst, cat):
        nc._cats[inst.ins.name] = cat
        return inst

    ctx_d = nc.declare_dram_parameter("ctx", [BL, S, D], dt.float32, isOutput=False)
    maskT_d = nc.declare_dram_parameter("maskT", [BL, 128, ST], dt.float32, isOutput=False)
    wct_d = nc.declare_dram_parameter("wct", [D, H], dt.float16, isOutput=False)
    wit_d = nc.declare_dram_parameter("wit", [D, H], dt.float16, isOutput=False)
    inpT_d = nc.declare_dram_parameter("inpT", [D, BL], dt.float16, isOutput=False)
    v_d = nc.declare_dram_parameter("v", [128, HT], dt.float16, isOutput=False)
    bc2_d = nc.declare_dram_parameter("bc2", [128, HT], dt.float32, isOutput=False)
    bcr_d = nc.declare_dram_parameter("bcr", [1, H], dt.float16, isOutput=False)
    hid_d = nc.declare_dram_parameter("hidden", [BL, H], dt.float32, isOutput=True)
    alp_d = nc.declare_dram_parameter("alpha", [BL, S], dt.float32, isOutput=True)

    with tile.TileContext(nc) as tc:
        with (
            tc.tile_pool(name="const", bufs=1) as cpool,
            tc.tile_pool(name="nat", bufs=3) as natpool,
            tc.tile_pool(name="work", bufs=2) as wpool,
            tc.tile_pool(name="ctxT", bufs=4) as ctxTpool,
            tc.tile_pool(name="th", bufs=3) as thpool,
            tc.tile_pool(name="psT", bufs=2, space="PSUM") as psTpool,
            tc.tile_pool(name="mm", bufs=3, space="PSUM") as mmpool,
            tc.tile_pool(name="attc", bufs=1, space="PSUM") as attcpool,
            tc.tile_pool(name="tail", bufs=2, space="PSUM") as tailpool,
        ):
            def emit_nat_dmas(b):
                nats = []
                for q in range(NQ):
                    natq = natpool.tile([128, QT, D], dt.float16, tag=f"nat{q}")
                    nc.gpsimd.dma_start(
                        natq[:],
                        ctx_d.ap()[b, 128 * QT * q:128 * QT * (q + 1), :]
                        .rearrange("(t p) d -> p t d", p=128),
                    )
                    nats.append(natq)
                return nats

            # ---- constants ----
            id16 = cpool.tile([128, 128], dt.float16, tag="id16")
            make_identity(nc, id16[:])

            # get the first context load moving before the other constants;
            # batch 0's first quarter is split so transposes can start sooner
            def emit_nat_dmas_first():
                nats = []
                for q in range(NQ):
                    natq = natpool.tile([128, QT, D], dt.float16, tag=f"nat{q}",
                                        name=f"nat{q}")
                    if q == 0:
                        half = QT // 2
                        for hh in range(2):
                            nc.gpsimd.dma_start(
                                natq[:, half * hh:half * (hh + 1), :],
                                ctx_d.ap()[0, 128 * half * hh:128 * half * (hh + 1), :]
                                .rearrange("(t p) d -> p t d", p=128),
                            )
                    else:
                        nc.gpsimd.dma_start(
                            natq[:],
                            ctx_d.ap()[0, 128 * QT * q:128 * QT * (q + 1), :]
                            .rearrange("(t p) d -> p t d", p=128),
                        )
                    nats.append(natq)
                return nats

            nats_next = emit_nat_dmas_first()
            id32f = cpool.tile([32, 32], dt.float32, tag="id32f")
            make_identity(nc, id32f[:])
            id4 = cpool.tile([4, 4], dt.float16, tag="id4")
            make_identity(nc, id4[:])
            id128f = cpool.tile([128, 128], dt.float32, tag="id128f")
            make_identity(nc, id128f[:])
            ones_row = cpool.tile([1, 128], dt.float16, tag="ones_row")
            nc.gpsimd.memset(ones_row[:], 1.0)
            ones_col = cpool.tile([128, 1], dt.float16, tag="ones_col")
            nc.gpsimd.memset(ones_col[:], 1.0)

            wct_sb = cpool.tile([128, DT, H], dt.float16, tag="wct")
            nc.sync.dma_start(wct_sb[:], wct_d.ap().rearrange("(t p) h -> p t h", p=128))
            wit_sb = cpool.tile([128, DT, H], dt.float16, tag="wit")
            nc.sync.dma_start(wit_sb[:], wit_d.ap().rearrange("(t p) h -> p t h", p=128))
            inpT_sb = cpool.tile([128, DT, BL], dt.float16, tag="inpT")
            nc.sync.dma_start(inpT_sb[:], inpT_d.ap().rearrange("(t p) b -> p t b", p=128))
            v_sb = cpool.tile([128, HT], dt.float16, tag="v")
            nc.sync.dma_start(v_sb[:], v_d.ap())
            bc2_sb = cpool.tile([128, HT], dt.float32, tag="bc2")
            nc.sync.dma_start(bc2_sb[:], bc2_d.ap())
            bcr_sb = cpool.tile([1, H], dt.float16, tag="bcr")
            nc.sync.dma_start(bcr_sb[:], bcr_d.ap())
            maskT_sb = cpool.tile([128, BL, ST], dt.float32, tag="maskT")
            nc.sync.dma_start(maskT_sb[:], maskT_d.ap().rearrange("b p k -> p b k"))

            # ---- bias_sb[p, ht, b] = inp[b, 128*ht+p] + b_in[.] + b_ctx[.]
            # (emitted after the first transposes so PE can start immediately)
            bias_sb = cpool.tile([128, HT, BL], dt.float32, tag="bias")

            def emit_bias():
                bias_ps = tailpool.tile([128, HT, BL], dt.float32, tag="tail", name="bias_ps")
                for mt in range(HT):
                    for kt in range(DT):
                        nc.tensor.matmul(
                            bias_ps[:, mt, :],
                            wit_sb[:, kt, 128 * mt:128 * (mt + 1)],
                            inpT_sb[:, kt, :],
                            start=(kt == 0), stop=(kt == DT - 1),
                        )
                for mt in range(HT):
                    nc.scalar.activation(
                        bias_sb[:, mt, :], bias_ps[:, mt, :],
                        AF.Identity, bias=bc2_sb[:, mt:mt + 1],
                    )

            wcnT_all = cpool.tile([128, DT, BL], dt.float16, tag="wcnT")

            # ---- deferred softmax/weighted-sum tail, emitted in segments ----
            def tail_seg0(st):
                b = st["b"]
                st["attn32"] = wpool.tile([32, 128], dt.float32, tag="attn32", name="attn32")
                nc.sync.dma_start(st["attn32"][:], st["att_row"][0:1, :])
                st["attT_ps"] = tailpool.tile([128, 32], dt.float32, tag="tail", name="attT_ps")
                nc.tensor.transpose(st["attT_ps"][:], st["attn32"][:], id32f[:])
                st["attm"] = wpool.tile([128, ST], dt.float32, tag="attm", name="attm")
                colmax = wpool.tile([128, 1], dt.float32, tag="colmax")
                nc.vector.tensor_tensor(
                    st["attm"][:], st["attT_ps"][:], maskT_sb[:, b, :],
                    mybir.AluOpType.add,
                )
                nc.vector.tensor_reduce(
                    colmax[:], st["attm"][:], axis=mybir.AxisListType.X,
                    op=mybir.AluOpType.max,
                )
                st["colmax16"] = wpool.tile([128, 1], dt.float16, tag="colmax16", name="colmax16")
                nc.vector.tensor_copy(st["colmax16"][:], colmax[:])

            def tail_seg1(st):
                cm_ps = tailpool.tile([1, 128], dt.float16, tag="tail")
                nc.tensor.transpose(cm_ps[:], st["colmax16"][:], id16[:])
                negmax16 = wpool.tile([1, 1], dt.float16, tag="negmax16")
                nc.vector.tensor_reduce(
                    negmax16[:], cm_ps[:], axis=mybir.AxisListType.X,
                    op=mybir.AluOpType.max, negate=True,
                )
                nm_ps = tailpool.tile([128, 1], dt.float32, tag="tail")
                nc.tensor.matmul(nm_ps[:], ones_row[:], negmax16[:], start=True, stop=True)
                st["nm_sb"] = wpool.tile([128, 1], dt.float32, tag="nm_sb", name="nm_sb")
                nc.vector.tensor_copy(st["nm_sb"][:], nm_ps[:])
                st["exp_t"] = wpool.tile([128, ST], dt.float16, tag="exp_t", name="exp_t")
                nc.scalar.activation(st["exp_t"][:], st["attm"][:], AF.Exp, bias=st["nm_sb"][:])

            def tail_seg2(st):
                cs_ps = tailpool.tile([1, ST], dt.float32, tag="tail")
                nc.tensor.matmul(cs_ps[:], ones_col[:], st["exp_t"][:], start=True, stop=True)
                sumexp = wpool.tile([1, 1], dt.float32, tag="sumexp")
                nc.vector.tensor_reduce(
                    sumexp[:], cs_ps[:], axis=mybir.AxisListType.X,
                    op=mybir.AluOpType.add,
                )
                st["recip"] = wpool.tile([1, 1], dt.float32, tag="recip", name="recip")
                nc.vector.reciprocal(st["recip"][:], sumexp[:])
                st["recip16"] = wpool.tile([1, 1], dt.float16, tag="recip16", name="recip16")
                nc.vector.tensor_copy(st["recip16"][:], st["recip"][:])

            def tail_seg3(st):
                b, nats, exp_t = st["b"], st["nats"], st["exp_t"]
                et_ps = tailpool.tile([32, 128], dt.float16, tag="tail")
                nc.tensor.transpose(et_ps[:], exp_t[:], id16[:])
                rb_ps = tailpool.tile([128, 1], dt.float32, tag="tail")
                nc.tensor.matmul(rb_ps[:], ones_row[:], st["recip16"][:], start=True, stop=True)
                alpha32 = wpool.tile([32, 128], dt.float32, tag="alpha32")
                nc.vector.tensor_scalar(
                    out=alpha32[:], in0=et_ps[:],
                    scalar1=rb_ps[0:32, :], scalar2=1e-6,
                    op0=mybir.AluOpType.mult, op1=mybir.AluOpType.add,
                )
                nc.sync.dma_start(
                    alp_d.ap()[b].rearrange("(k p) -> k p", k=32), alpha32[:]
                )
                # weighted context: wc = sum_s exp[s] * context[b, s, :]
                wc_ps = tailpool.tile([1, D], dt.float32, tag="tail")
                for sti in range(ST):
                    reg(nc.tensor.matmul(
                        wc_ps[:], exp_t[:, sti:sti + 1], nats[sti // QT][:, sti % QT, :],
                        start=(sti == 0), stop=(sti == ST - 1),
                    ), "wsum")
                wcn16 = wpool.tile([1, D], dt.float16, tag="wcn16")
                nc.vector.tensor_scalar(
                    out=wcn16[:], in0=wc_ps[:], scalar1=st["recip"][:], scalar2=None,
                    op0=mybir.AluOpType.mult,
                )
                wcn4 = wpool.tile([4, 128], dt.float16, tag="wcn4")
                nc.sync.dma_start(wcn4[:], wcn16[:])
                wcnT_ps = tailpool.tile([128, 4], dt.float16, tag="tail")
                nc.tensor.transpose(wcnT_ps[:], wcn4[:], id4[:])
                nc.vector.tensor_copy(wcnT_all[:, :, b], wcnT_ps[:])

            tail_segs = [tail_seg0, tail_seg1, tail_seg2, tail_seg3]
            pending = None

            for b in range(BL):
                nats = nats_next
                att_row = wpool.tile([1, S], dt.float32, tag="att_row")

                def emit_att_group(sc, th):
                    attc = attcpool.tile([1, 512], dt.float32, tag="attc", name="attc")
                    for ht in range(HT):
                        reg(nc.tensor.matmul(
                            attc[:], v_sb[:, ht:ht + 1], th[:, ht, :],
                            start=(ht == 0), stop=(ht == HT - 1),
                        ), "att")
                    nc.vector.tensor_copy(att_row[0:1, 512 * sc:512 * (sc + 1)], attc[:])

                pending_att = None
                for sc in range(SC):
                    # transpose 4 s-tiles x 4 d-blocks -> ctxT [128d, DT, 512s]
                    ctxT = ctxTpool.tile([128, DT, 512], dt.float16, tag="ctxT")
                    for dtb in range(DT):
                        psT = psTpool.tile([128, 512], dt.float16, tag="psT")
                        for j in range(4):
                            sti = 4 * sc + j
                            reg(nc.tensor.transpose(
                                psT[:, 128 * j:128 * (j + 1)],
                                nats[sti // QT][:, sti % QT, 128 * dtb:128 * (dtb + 1)],
                                id16[:],
                            ), "T")
                        nc.vector.tensor_copy(ctxT[:, dtb, :], psT[:])

                    if b == 0 and sc == 0:
                        emit_bias()

                    th = thpool.tile([128, HT, 512], dt.float16, tag="th")
                    for ht in range(HT):
                        mmps = mmpool.tile([128, 512], dt.float32, tag="mm")
                        for dtb in range(DT):
                            reg(nc.tensor.matmul(
                                mmps[:],
                                wct_sb[:, dtb, 128 * ht:128 * (ht + 1)],
                                ctxT[:, dtb, :],
                                start=(dtb == 0), stop=(dtb == DT - 1),
                            ), "main")
                        nc.scalar.activation(
                            th[:, ht, :], mmps[:], AF.Tanh,
                            bias=bias_sb[:, ht, b:b + 1],
                        )
                    if pending_att is not None:
                        emit_att_group(*pending_att)
                    pending_att = (sc, th)

                    # interleave the previous batch's softmax/wsum tail
                    if pending is not None and 1 <= sc <= len(tail_segs):
                        tail_segs[sc - 1](pending)
                        if sc == len(tail_segs):
                            pending = None
                    if sc == 1 and b + 1 < BL:
                        nats_next = emit_nat_dmas(b + 1)

                emit_att_group(*pending_att)
                pending = {"b": b, "nats": nats, "att_row": att_row}

            for seg in tail_segs:
                seg(pending)

            # ---- hidden = W_ctx @ wcn + b_ctx, all 4 batches at once ----
            hid_ps = tailpool.tile([128, HT, BL], dt.float32, tag="tail")
            for mt in range(HT):
                for kt in range(DT):
                    nc.tensor.matmul(
                        hid_ps[:, mt, :],
                        wct_sb[:, kt, 128 * mt:128 * (mt + 1)],
                        wcnT_all[:, kt, :],
                        start=(kt == 0), stop=False,
                    )
                nc.tensor.matmul(
                    hid_ps[:, mt, :], bcr_sb[0:1, 128 * mt:128 * (mt + 1)],
                    ones_row[0:1, 0:BL],
                    start=False, stop=True,
                )
            hid_sb = wpool.tile([128, HT, BL], dt.float32, tag="hid_sb")
            nc.vector.tensor_copy(hid_sb[:], hid_ps[:])
            hidT_ps = tailpool.tile([HT * BL, 128], dt.float32, tag="tail")
            nc.tensor.transpose(
                hidT_ps[:], hid_sb[:].rearrange("p t b -> p (t b)"), id128f[:]
            )
            hidT_sb = wpool.tile([HT * BL, 128], dt.float32, tag="hidT_sb")
            nc.vector.tensor_copy(hidT_sb[:], hidT_ps[:])
            nc.sync.dma_start(
                hid_d.ap().rearrange("b (t p) -> t b p", p=128), hidT_sb[:]
            )

    nc.finalize()
    return nc


def _get_nc():
    if not _NC_CACHE:
        _NC_CACHE.append(_build_nc())
    return _NC_CACHE[0]


def _prepare_in_maps(input, context, mask, W_in, b_in, W_ctx, b_ctx, V):
    input = np.asarray(input, dtype=np.float32)
    context = np.asarray(context, dtype=np.float32)
    mask = np.asarray(mask)
    W_in = np.asarray(W_in, dtype=np.float32)
    b_in = np.asarray(b_in, dtype=np.float32)
    W_ctx = np.asarray(W_ctx, dtype=np.float32)
    b_ctx = np.asarray(b_ctx, dtype=np.float32)
    V = np.asarray(V, dtype=np.float32)

    # host-side prep (small tensors only)
    maskadd = np.where(mask, np.float32(-1e30), np.float32(0.0))      # [B, S]
    maskT = np.ascontiguousarray(
        maskadd.reshape(B, ST, 128).transpose(0, 2, 1))               # [B, 128, ST]
    wct16 = np.ascontiguousarray(W_ctx.T).astype(np.float16)          # [D, H]
    wit16 = np.ascontiguousarray(W_in.T).astype(np.float16)           # [D, H]
    v16 = np.ascontiguousarray(V.reshape(HT, 128).T).astype(np.float16)   # [128, HT]
    bc2 = np.ascontiguousarray((b_in + b_ctx).reshape(HT, 128).T)     # [128, HT] f32
    bcr16 = b_ctx.reshape(1, H).astype(np.float16)                    # [1, H]

    in_maps = []
    for c in range(NCORES):
        bs = slice(BL * c, BL * (c + 1))
        in_maps.append({
            "ctx": context[bs],
            "maskT": maskT[bs],
            "wct": wct16,
            "wit": wit16,
            "inpT": np.ascontiguousarray(input[bs].T).astype(np.float16),
            "v": v16,
            "bc2": bc2,
            "bcr": bcr16,
        })
    return in_maps


def kernel(input, context, mask, W_in, b_in, W_ctx, b_ctx, V, **run_kwargs):
    from concourse.bass_utils import run_bass_kernel_spmd

    nc = _get_nc()
    in_maps = _prepare_in_maps(input, context, mask, W_in, b_in, W_ctx, b_ctx, V)
    res = run_bass_kernel_spmd(nc, in_maps, list(range(NCORES)), **run_kwargs)
    hidden = np.concatenate([res.results[c]["hidden"] for c in range(NCORES)], axis=0)
    alpha = np.concatenate([res.results[c]["alpha"] for c in range(NCORES)], axis=0)
    if run_kwargs:
        kernel.last_result = res
    return hidden, alpha
